# revision 20
# baseline (speedup 1.0000x reference)
"""Multi-head self-attention (B=2, S=2048, E=1024, H=16, D=64) on 8 NeuronCores.

Sharding: core c -> (batch b = c // 4, head group g = c % 4).  Each core
computes Q/K/V projections for its 4 heads (column-parallel), attention, and
per-head-pair partial output projections (row-parallel); the host sums the 8
partials per batch.  Device activations live in "transposed space" (feature
on the partition dim) so every matmul contracts along partitions:

  Q^T = Wq_g^T @ X^T          [256, 2048]  (bias folded into the psum copy)
  K^T = Wk_g^T @ X^T          [256, 2048]
  V   = X @ Wv_g              [2048, 256]  (projected pre-stream, all 8 banks)
  S^T = K_h @ Q_h^T           per head, row-packed head pairs
  P^T = exp(S^T * scale)      one ACTIVATE per 3 psum banks (N=1536)
  O'^T = V2^T @ P^T           col-packed pair (concurrent 64-col tiles)
  r    = ones32^T @ P^T       col-tiled M=32 matmuls -> denominators
                              replicated over 32 partitions (no broadcast)
  O^T  = O'^T * (1/r)         DVE reciprocal + one full-width multiply
  Y^T[hc] = Wo_hc^T @ O^T[hc] [1024, 2048] fp16 per-pair partial

The schedule is a software pipeline driven by the ScalarE exp stream (the
roofline engine at ~128us busy): a ring of 2x[128,3,512] psum tiles feeds
N=1536 ACTIVATEs back-to-back, while PV / projections / output chunks fill
the PE's leftover cycles in emission-priority order.  Score-pair matmuls and
the PV pair are emitted adjacently so the PE runs them as concurrent
row/col-tiled 64-wide tiles.
"""

from contextlib import ExitStack

import numpy as np

import concourse.bass as bass
import concourse.tile as tile
from concourse import bacc, mybir
from concourse.bass_utils import run_bass_kernel_spmd

B, S, E, H, D = 2, 2048, 1024, 16, 64
NCORES = 8
GH = 4            # heads per core
DC = GH * D       # head-dim columns per core (256)
EC = E // 128     # 8 e-chunks
KC = S // 128     # 16 k-chunks
QC = S // 512     # 4 q-chunks
F32 = mybir.dt.float32
MM_DT = mybir.dt.float16
EXP_FUNC = mybir.ActivationFunctionType.Exp
SCALE = 1.0 / np.sqrt(np.float32(D))

# (qc, hc) block order: hc-inner so y(qc) partials become eligible early and
# the drain only carries the last half-block's output chunk.
BLOCK_ORDER = [(q, h) for q in range(QC) for h in range(2)]
NBLK = len(BLOCK_ORDER)
SLICES_PER_BLOCK = 2 * KC          # (kc, hp) pairs
NSLICE = NBLK * SLICES_PER_BLOCK   # 256
TW = 3                             # slices per exp tile (psum banks/ACTIVATE)
NTILE = (NSLICE + TW - 1) // TW


def round_f32r(a):
    return np.ascontiguousarray(a, np.float32).astype(np.float16)


def slice_info(s):
    b = s // SLICES_PER_BLOCK
    w = s % SLICES_PER_BLOCK
    kc, hp = w // 2, w % 2
    qc, hc = BLOCK_ORDER[b]
    return b, kc, hp, qc, hc


DEBUG_DUMPS = False


def _emit(nc, tc, ctx, xT, wq, wk, wv, wo, bq, bk, yT, dbg=None):
    sb_big = ctx.enter_context(tc.tile_pool(name="sb_big", bufs=1))
    sb_p = ctx.enter_context(tc.tile_pool(name="sb_p", bufs=16))
    sb_norm = ctx.enter_context(tc.tile_pool(name="sb_norm", bufs=2))
    sb_y = ctx.enter_context(tc.tile_pool(name="sb_y", bufs=4))
    ps_ring = ctx.enter_context(tc.tile_pool(name="ps_ring", bufs=2, space="PSUM"))
    ps_acc = ctx.enter_context(tc.tile_pool(name="ps_acc", bufs=1, space="PSUM"))
    ps_flex = ctx.enter_context(tc.tile_pool(name="ps_flex", bufs=1, space="PSUM"))

    xT_t = sb_big.tile([128, EC, S], MM_DT)
    wq_t = sb_big.tile([128, EC, DC], MM_DT)
    wk_t = sb_big.tile([128, EC, DC], MM_DT)
    wv_t = sb_big.tile([128, EC, DC], MM_DT)
    wo_t = sb_big.tile([128, 2, E], MM_DT)
    bq_t = sb_big.tile([128, 2], F32)
    bk_t = sb_big.tile([128, 2], F32)
    ones32 = sb_big.tile([128, 32], MM_DT)
    warm = sb_big.tile([1, 8], F32)
    qT_t = sb_big.tile([128, 2, S], MM_DT)
    kT_t = sb_big.tile([128, 2, S], MM_DT)
    v_t = sb_big.tile([128, KC, 2, 128], MM_DT)
    o_t = sb_big.tile([128, 2, S], MM_DT)

    # ACT table preload: a dummy exp so the ~2.7us exp_and_others table DMA
    # happens during the input DMA, not before the first real ACTIVATE.
    nc.vector.memset(warm[:, :], 0.0)
    nc.scalar.activation(out=warm[:, :], in_=warm[:, :], func=EXP_FUNC)
    nc.vector.memset(ones32[:, :], 1.0)

    # Input DMAs.  bq/bk lead the scalar ring (needed by the first psum
    # copies); weights next; the xT chunks are split across both HWDGE rings.
    nc.scalar.dma_start(out=bq_t[:, :], in_=bq)
    nc.scalar.dma_start(out=bk_t[:, :], in_=bk)
    nc.scalar.dma_start(out=wq_t[:, :, :],
                        in_=wq.rearrange("p (c d) -> p c d", c=EC))
    nc.scalar.dma_start(out=wk_t[:, :, :],
                        in_=wk.rearrange("p (c d) -> p c d", c=EC))
    nc.scalar.dma_start(out=wv_t[:, :, :],
                        in_=wv.rearrange("p (c d) -> p c d", c=EC))
    for ec in range(EC):
        eng = nc.sync if ec < 5 else nc.scalar
        eng.dma_start(out=xT_t[:, ec, :], in_=xT[:, ec * S:(ec + 1) * S])
    nc.scalar.dma_start(out=wo_t[:, :, :],
                        in_=wo.rearrange("p (c e) -> p c e", c=2))

    # ---- pre-stream V projection ---------------------------------------
    # All 8 psum banks act as V accumulators while the input DMA streams:
    # ring tile A -> kc pairs 0-2, ring tile B -> 3-5, acc -> 6, flex -> 7.
    ringA = ps_ring.tile([128, TW, 512], F32, tag="ring", name="v_ringA")
    ringB = ps_ring.tile([128, TW, 512], F32, tag="ring", name="v_ringB")
    accV = ps_acc.tile([128, 512], F32, tag="bank", name="v_acc")
    flexV = ps_flex.tile([128, 512], F32, tag="bank", name="v_flex")

    def v_bank(kcp):
        if kcp < 3:
            return ringA[:, kcp, :]
        if kcp < 6:
            return ringB[:, kcp - 3, :]
        return accV[:, :] if kcp == 6 else flexV[:, :]

    # start=True clears the whole psum bank row, so only the FIRST matmul
    # emitted per bank may carry it; the second column-half group relies on
    # the cleared has_written bits (first write per element = overwrite).
    for ec in range(EC):
        for kcp in range(KC // 2):
            for half in range(2):
                kc = 2 * kcp + half
                nc.tensor.matmul(
                    v_bank(kcp)[:, half * 256:half * 256 + 256],
                    lhsT=xT_t[:, ec, kc * 128:(kc + 1) * 128],
                    rhs=wv_t[:, ec, :],
                    start=(ec == 0 and half == 0), stop=(ec == EC - 1))
    for kcp in range(KC // 2):
        for half in range(2):
            kc = 2 * kcp + half
            nc.vector.tensor_copy(
                out=v_t[:, kc, :, :],
                in_=v_bank(kcp)[:, half * 256:half * 256 + 256].rearrange(
                    "p (h d) -> p h d", h=2))

    # ---- helpers -------------------------------------------------------
    def qk_half(hc, proj, sc, half, state={}):
        w_t, dst, b_t = ((wq_t, qT_t, bq_t), (wk_t, kT_t, bk_t))[proj]
        key = (hc, proj, sc)
        if half == 0:
            state[key] = ps_flex.tile([128, 512], F32, tag="bank", name="ps_qk")
        ps = state[key]
        for ec in (range(EC // 2) if half == 0 else range(EC // 2, EC)):
            nc.tensor.matmul(
                ps[:, :],
                lhsT=w_t[:, ec, hc * 128:(hc + 1) * 128],
                rhs=xT_t[:, ec, sc * 512:(sc + 1) * 512],
                start=(ec == 0), stop=(ec == EC - 1))
        if half == 1:
            nc.vector.tensor_scalar_add(
                out=dst[:, hc, sc * 512:(sc + 1) * 512],
                in0=ps[:, :], scalar1=b_t[:, hc:hc + 1])
            del state[key]

    def y_half(qc, hc, ec, pool):
        yp = pool.tile([128, 512], F32, tag="bank", name="ps_y")
        nc.tensor.matmul(
            yp[:, :],
            lhsT=wo_t[:, hc, ec * 128:(ec + 1) * 128],
            rhs=o_t[:, hc, qc * 512:(qc + 1) * 512],
            start=True, stop=True)
        ys = sb_y.tile([128, 512], MM_DT)
        nc.vector.tensor_copy(out=ys[:, :], in_=yp[:, :])
        nc.sync.dma_start(
            out=yT[hc * E + ec * 128:hc * E + (ec + 1) * 128,
                   qc * 512:(qc + 1) * 512],
            in_=ys[:, :])

    pt_ap = [None] * NSLICE

    def scores_mm(ring, j, s):
        _, kc, hp, qc, hc = slice_info(s)
        po = hp * 64
        nc.tensor.matmul(
            ring[:, j, :],
            lhsT=kT_t[po:po + 64, hc, kc * 128:(kc + 1) * 128],
            rhs=qT_t[po:po + 64, hc, qc * 512:(qc + 1) * 512],
            start=True, stop=True)

    acc_state = {}

    def pv_pair(b, kc):
        qc, hc = BLOCK_ORDER[b]
        if b not in acc_state:
            acc_state[b] = ps_acc.tile([128, 512], F32, tag="bank", name="acc")
        acc = acc_state[b]
        for hp in range(2):
            pt, j = pt_ap[b * SLICES_PER_BLOCK + 2 * kc + hp]
            nc.tensor.matmul(
                acc[hp * 64:hp * 64 + 64, :],
                lhsT=v_t[:, kc, hc, hp * 64:hp * 64 + 64],
                rhs=pt[:, j, :],
                start=(kc == 0), stop=(kc == KC - 1))

    dn_state = {}

    def denom_quads(b, q0, nq):
        # lanes: 0 = hp0/kc0-7, 1 = hp0/kc8-15, 2 = hp1/kc0-7, 3 = hp1/kc8-15
        if b not in dn_state:
            dn_state[b] = ps_flex.tile([128, 512], F32, tag="bank", name="ps_dn")
        dn = dn_state[b]
        for step in range(q0, q0 + nq):
            for lane in range(4):
                hp, hkc = lane // 2, lane % 2
                kc = hkc * 8 + step
                pt, j = pt_ap[b * SLICES_PER_BLOCK + 2 * kc + hp]
                nc.tensor.matmul(
                    dn[32 * lane:32 * lane + 32, :],
                    lhsT=ones32[:, :],
                    rhs=pt[:, j, :],
                    start=(step == 0), stop=(step == 7),
                    tile_position=(0, 32 * lane))

    def norm_block(b):
        qc, hc = BLOCK_ORDER[b]
        acc = acc_state.pop(b)
        dn = dn_state.pop(b)
        tmp = sb_norm.tile([128, 512], F32, tag="tmp")
        inv = sb_norm.tile([128, 512], F32, tag="inv")
        nc.vector.tensor_copy(out=tmp[:, :], in_=dn[:, :])
        if dbg is not None:
            nc.sync.dma_start(out=dbg["dn"][b * 128:(b + 1) * 128, :],
                              in_=tmp[:, :])
            accd = sb_norm.tile([128, 512], F32, tag="accd")
            nc.vector.tensor_copy(out=accd[:, :], in_=acc[:, :])
            nc.sync.dma_start(out=dbg["accd"][b * 128:(b + 1) * 128, :],
                              in_=accd[:, :])
        # r_hp = lane(2hp) + lane(2hp+1), replicated into both 32-row strips.
        # Mixed PSUM+SBUF operands may use different base partitions (the
        # equal-base rule only binds SB+SB pairs).
        for hp in range(2):
            base = 64 * hp
            for sub in range(2):
                nc.vector.tensor_add(
                    out=inv[base + 32 * sub:base + 32 * sub + 32, :],
                    in0=dn[base + 32 * (1 - sub):base + 32 * (2 - sub), :],
                    in1=tmp[base + 32 * sub:base + 32 * sub + 32, :])
        nc.vector.reciprocal_approx_fast(out=inv[:, :], in_=inv[:, :])
        nc.vector.tensor_mul(
            o_t[:, hc, qc * 512:(qc + 1) * 512], acc[:, :], inv[:, :])

    # ---- startup projections -------------------------------------------
    qk_half(0, 1, 0, 0)
    qk_half(0, 1, 0, 1)   # kT hc0 sc0 on flex (after v_flex copy releases)
    qk_half(0, 0, 0, 0)
    qk_half(0, 0, 0, 1)   # qT hc0 qc0 (flex again; acc reserved for PV b0)

    # ---- filler schedule: tile -> closures ------------------------------
    fillers = {}

    def put(t, fn):
        fillers.setdefault(t, []).append(fn)

    def qkf(hc, proj, sc, half):
        return lambda: qk_half(hc, proj, sc, half)

    qk_seq = [  # (tile, hc, proj, sc) halves at t, t+1; deadlines in comments
        (0, 0, 1, 1),    # kT(0,s1) by t2.7
        (2, 0, 1, 2),    # by t5.3
        (4, 0, 1, 3),    # by t8
        (6, 1, 1, 0),    # kT(1,s0) by t10.7
        (8, 1, 0, 0),    # qT(1,0) by t10.7
        (10, 1, 1, 1),   # by t13.3
        (12, 1, 1, 2),   # by t16
        (14, 1, 1, 3),   # by t18.7
        (16, 0, 0, 1),   # qT(0,1) by t21.3
        (18, 1, 0, 1),   # qT(1,1) by t21.3
        (24, 0, 0, 2),   # by t42.7
        (26, 1, 0, 2),
        (30, 0, 0, 3),   # by t64
        (32, 1, 0, 3),
    ]
    for t0, hc, proj, sc in qk_seq:
        put(t0, qkf(hc, proj, sc, 0))
        put(t0 + 1, qkf(hc, proj, sc, 1))

    # ---- main stream ----------------------------------------------------
    pv_done = 0
    dn_done = [0] * NBLK      # quads emitted per block
    normed = [False] * NBLK
    y_queue = []              # pending (qc, hc, ec)
    y_delay = {}              # block -> tile when norm emitted

    def emit_background(t):
        nonlocal pv_done
        exp_slices = min(NSLICE, max(0, (t - 1) * TW))
        budget = 3
        while budget > 0 and pv_done < NBLK * KC:
            b, kc = pv_done // KC, pv_done % KC
            if (b * KC + kc + 1) * 2 > exp_slices:
                break
            if kc == 0 and b > 0 and not normed[b - 1]:
                break
            pv_pair(b, kc)
            pv_done += 1
            budget -= 1
        # denominator quads: 2 per call once the block's slices are exp'd
        for b in range(NBLK):
            if dn_done[b] < 8 and (b + 1) * SLICES_PER_BLOCK <= exp_slices:
                nq = min(2, 8 - dn_done[b])
                denom_quads(b, dn_done[b], nq)
                dn_done[b] += nq
                break
        # norm once denominators + PV of a block are complete
        for b in range(NBLK):
            if dn_done[b] == 8 and not normed[b] and pv_done >= (b + 1) * KC:
                norm_block(b)
                normed[b] = True
                y_delay[b] = t
                qc, hc = BLOCK_ORDER[b]
                for ec in range(EC):
                    y_queue.append((qc, hc, ec))
                break
        # one y chunk per call, 2 tiles after its norm
        if y_queue:
            qc, hc, ec = y_queue[0]
            b = BLOCK_ORDER.index((qc, hc))
            if t >= y_delay[b] + 2:
                y_queue.pop(0)
                y_half(qc, hc, ec, ps_flex)

    # Pair-aligned emission: process tiles in groups of two (6 slices = 3
    # score pairs) so both matmuls of every pair are adjacent -> the PE runs
    # them as concurrent 64-row tiles.
    t = 0
    while t < NTILE:
        n_a = min(TW, NSLICE - t * TW)
        n_b = min(TW, max(0, NSLICE - (t + 1) * TW))
        ring_a = ps_ring.tile([128, TW, 512], F32, tag="ring", name="ring_a")
        pt_a = sb_p.tile([128, TW, 512], MM_DT, tag="pt", name="pt_a")
        ring_b = pt_b = None
        if n_b:
            ring_b = ps_ring.tile([128, TW, 512], F32, tag="ring", name="ring_b")
            pt_b = sb_p.tile([128, TW, 512], MM_DT, tag="pt", name="pt_b")

        def emit_slice(j):
            if j < n_a:
                s = t * TW + j
                scores_mm(ring_a, j, s)
                pt_ap[s] = (pt_a, j)
            elif j - n_a < n_b:
                s = (t + 1) * TW + (j - n_a)
                scores_mm(ring_b, j - n_a, s)
                pt_ap[s] = (pt_b, j - n_a)

        # pairs 0-1 (slices 0-3), exp_a, pair 2 (slices 4-5), exp_b
        for j in range(min(4, n_a + n_b)):
            emit_slice(j)
        nc.scalar.activation(
            out=pt_a[:, 0:n_a, :], in_=ring_a[:, 0:n_a, :], func=EXP_FUNC,
            scale=float(SCALE))
        for fn in fillers.get(t, []):
            fn()
        emit_background(t)
        if n_b:
            for j in range(4, n_a + n_b):
                emit_slice(j)
            nc.scalar.activation(
                out=pt_b[:, 0:n_b, :], in_=ring_b[:, 0:n_b, :], func=EXP_FUNC,
                scale=float(SCALE))
            for fn in fillers.get(t + 1, []):
                fn()
            emit_background(t + 1)
        t += 2

    # ---- drain ----------------------------------------------------------
    t = NTILE + 2
    while pv_done < NBLK * KC or not all(normed) or y_queue:
        emit_background(t)
        t += 1
        if t > NTILE + 200:
            raise RuntimeError("drain did not converge")

    if dbg is not None:
        for name, src in (("qT", qT_t), ("kT", kT_t), ("o", o_t)):
            nc.sync.dma_start(out=dbg[name],
                              in_=src.rearrange("p a b -> p (a b)"))
        nc.sync.dma_start(out=dbg["v"],
                          in_=v_t.rearrange("p a b c -> p (a b c)"))


_cached_nc = None


def _build():
    nc = bacc.Bacc(trn_type="TRN2", target_bir_lowering=False)
    xT = nc.dram_tensor("xT", [128, EC * S], MM_DT, kind="ExternalInput").ap()
    wq = nc.dram_tensor("wq", [128, EC * DC], MM_DT, kind="ExternalInput").ap()
    wk = nc.dram_tensor("wk", [128, EC * DC], MM_DT, kind="ExternalInput").ap()
    wv = nc.dram_tensor("wv", [128, EC * DC], MM_DT, kind="ExternalInput").ap()
    wo = nc.dram_tensor("wo", [128, 2 * E], MM_DT, kind="ExternalInput").ap()
    bq = nc.dram_tensor("bq", [128, 2], F32, kind="ExternalInput").ap()
    bk = nc.dram_tensor("bk", [128, 2], F32, kind="ExternalInput").ap()
    yT = nc.dram_tensor("yT", [2 * E, S], MM_DT, kind="ExternalOutput").ap()
    dbg = None
    if DEBUG_DUMPS:
        dbg = {
            "qT": nc.dram_tensor("dbg_qT", [128, 2 * S], MM_DT,
                                 kind="ExternalOutput").ap(),
            "kT": nc.dram_tensor("dbg_kT", [128, 2 * S], MM_DT,
                                 kind="ExternalOutput").ap(),
            "o": nc.dram_tensor("dbg_o", [128, 2 * S], MM_DT,
                                kind="ExternalOutput").ap(),
            "v": nc.dram_tensor("dbg_v", [128, KC * 2 * 128], MM_DT,
                                kind="ExternalOutput").ap(),
            "dn": nc.dram_tensor("dbg_dn", [NBLK * 128, 512], F32,
                                 kind="ExternalOutput").ap(),
            "accd": nc.dram_tensor("dbg_accd", [NBLK * 128, 512], F32,
                                   kind="ExternalOutput").ap(),
        }
    with tile.TileContext(nc) as tc:
        with ExitStack() as ctx:
            _emit(nc, tc, ctx, xT, wq, wk, wv, wo, bq, bk, yT, dbg)
    nc.compile()
    return nc


def get_nc():
    global _cached_nc
    if _cached_nc is None:
        _cached_nc = _build()
    return _cached_nc


def perm(a):
    # [C*128, N] -> [128, C*N] with SBUF chunk-major free dim
    cN = a.shape[0] // 128
    return np.ascontiguousarray(
        a.reshape(cN, 128, a.shape[1]).transpose(1, 0, 2).reshape(
            128, cN * a.shape[1]))


def make_in_maps(inputs, wq, bq, wk, bk, wv, wo):
    in_maps = []
    for c in range(NCORES):
        b, g = divmod(c, GH)
        sl = slice(g * DC, (g + 1) * DC)
        in_maps.append({
            "xT": round_f32r(perm(np.ascontiguousarray(inputs[b].T))),
            "wq": round_f32r(perm(wq[:, sl])),
            "wk": round_f32r(perm(wk[:, sl])),
            "wv": round_f32r(perm(wv[:, sl])),
            "wo": round_f32r(perm(wo[sl, :])),
            "bq": np.ascontiguousarray(bq[sl].reshape(2, 128).T, np.float32),
            "bk": np.ascontiguousarray(bk[sl].reshape(2, 128).T, np.float32),
        })
    return in_maps


def combine(results, wv_full, bv, wo_full, bo):
    y = np.zeros((B, S, E), np.float32)
    for c in range(NCORES):
        yT = np.asarray(results[c]["yT"], np.float32)
        y[c // GH] += (yT[:E] + yT[E:]).T
    y += bv @ wo_full + bo
    return y


def kernel(inputs, wq, bq, wk, bk, wv, bv, wo, bo, _run_kwargs=None):
    inputs = np.asarray(inputs, np.float32)
    wq, bq = np.asarray(wq, np.float32), np.asarray(bq, np.float32)
    wk, bk = np.asarray(wk, np.float32), np.asarray(bk, np.float32)
    wv, bv = np.asarray(wv, np.float32), np.asarray(bv, np.float32)
    wo, bo = np.asarray(wo, np.float32), np.asarray(bo, np.float32)

    nc = get_nc()
    in_maps = make_in_maps(inputs, wq, bq, wk, bk, wv, wo)
    res = run_bass_kernel_spmd(nc, in_maps, list(range(NCORES)),
                               **(_run_kwargs or {}))
    y = combine(res.results, wv, bv, wo, bo)
    if _run_kwargs is not None:
        kernel.last_result = res
    return y


# revision 22
# speedup vs baseline: 1.0116x; 1.0116x over previous
"""Multi-head self-attention (B=2, S=2048, E=1024, H=16, D=64) on 8 NeuronCores.

Sharding: core c -> (batch b = c // 4, head group g = c % 4).  Each core
computes Q/K/V projections for its 4 heads (column-parallel), attention, and
per-head-pair partial output projections (row-parallel); the host sums the 8
partials per batch.  Device activations live in "transposed space" (feature
on the partition dim) so every matmul contracts along partitions:

  Q^T = Wq_g^T @ X^T          [256, 2048]  (bias folded into the psum copy)
  K^T = Wk_g^T @ X^T          [256, 2048]
  V   = X @ Wv_g              [2048, 256]  (mostly projected during input DMA)
  S^T = K_h @ Q_h^T           per head, row-packed head pairs
  P^T = exp(S^T * scale)      ACTIVATEs over 3- and 2-bank psum tiles
  O'^T = V2^T @ P^T           col-packed pair (concurrent 64-col tiles)
  r    = ones32^T @ P^T       col-tiled M=32 matmuls -> denominators
                              replicated over 32 partitions (no broadcast)
  O^T  = O'^T * (1/r)         DVE reciprocal + one full-width multiply
  Y^T[hc] = Wo_hc^T @ O^T[hc] [1024, 2048] fp16 per-pair partial

Schedule: a software pipeline driven by the ScalarE exp stream.  The exp ring
is an asymmetric 5-bank pair [A=3 banks, B=2 banks]; score pairs are permuted
so both matmuls of every (kc, head-pair) land adjacent in emission order and
run as concurrent 64-row PE tiles.  The other 3 psum banks are dedicated:
PV accumulator / qk-projection+denominators / output-projection chunks, so
no PE matmul ever head-blocks on an unrelated psum copy.  start=True clears
the whole psum bank row, so only the first matmul emitted per bank carries
it when two column-half groups share a bank.
"""

from contextlib import ExitStack

import numpy as np

import concourse.bass as bass
import concourse.tile as tile
from concourse import bacc, mybir
from concourse.bass_utils import run_bass_kernel_spmd

B, S, E, H, D = 2, 2048, 1024, 16, 64
NCORES = 8
GH = 4            # heads per core
DC = GH * D       # head-dim columns per core (256)
EC = E // 128     # 8 e-chunks
KC = S // 128     # 16 k-chunks
QC = S // 512     # 4 q-chunks
F32 = mybir.dt.float32
MM_DT = mybir.dt.float16
EXP_FUNC = mybir.ActivationFunctionType.Exp
SCALE = 1.0 / np.sqrt(np.float32(D))

# (qc, hc) block order: hc-inner so y(qc) partials become eligible early and
# the drain only carries the last half-block's output chunks.
BLOCK_ORDER = [(q, h) for q in range(QC) for h in range(2)]
NBLK = len(BLOCK_ORDER)
PAIRS_PER_BLOCK = KC
NPAIR = NBLK * PAIRS_PER_BLOCK     # 128 (block-major, kc-minor)
NSLICE = 2 * NPAIR

# Exp-ring windows: 5 pairs -> ring tiles [A(3 slices), B(2), A(3), B(2)]
# with pair p emitted adjacently: A=(p0,p0,p1) B=(p1,p2) A=(p2,p3,p3) B=(p4,p4)
WINDOW_PAIRS = 5


def round_f32r(a):
    return np.ascontiguousarray(a, np.float32).astype(np.float16)


def pair_info(p):
    b = p // PAIRS_PER_BLOCK
    kc = p % PAIRS_PER_BLOCK
    qc, hc = BLOCK_ORDER[b]
    return b, kc, qc, hc


def build_windows():
    """Yield exp-tile descriptors: (tag, [(pair, hp), ...]) in emission order.

    Every pair's two slices are adjacent in the global emission sequence,
    possibly straddling two consecutive tiles (safe: by emission time the
    earlier tile of the same tag has long been consumed).
    """
    seq = [(p, hp) for p in range(NPAIR) for hp in range(2)]
    tiles = []
    i = 0
    sizes = [3, 2]
    k = 0
    while i < len(seq):
        n = min(sizes[k % 2], len(seq) - i)
        tiles.append(("A" if k % 2 == 0 else "B", seq[i:i + n]))
        i += n
        k += 1
    return tiles


EXP_TILES = build_windows()
NEXP = len(EXP_TILES)
# cumulative slices after each exp tile
CUM_SLICES = []
_c = 0
for _tag, _sl in EXP_TILES:
    _c += len(_sl)
    CUM_SLICES.append(_c)


DEBUG_DUMPS = False


def _emit(nc, tc, ctx, xT, wq, wk, wv, wo, bq, bk, yT, dbg=None):
    sb_big = ctx.enter_context(tc.tile_pool(name="sb_big", bufs=1))
    sb_p = ctx.enter_context(tc.tile_pool(name="sb_p", bufs=10))
    sb_norm = ctx.enter_context(tc.tile_pool(name="sb_norm", bufs=2))
    sb_y = ctx.enter_context(tc.tile_pool(name="sb_y", bufs=4))
    ps_ring = ctx.enter_context(tc.tile_pool(name="ps_ring", bufs=1, space="PSUM"))
    ps_acc = ctx.enter_context(tc.tile_pool(name="ps_acc", bufs=1, space="PSUM"))
    ps_qk = ctx.enter_context(tc.tile_pool(name="ps_qk", bufs=1, space="PSUM"))
    ps_y = ctx.enter_context(tc.tile_pool(name="ps_y", bufs=1, space="PSUM"))

    xT_t = sb_big.tile([128, EC, S], MM_DT)
    wq_t = sb_big.tile([128, EC, DC], MM_DT)
    wk_t = sb_big.tile([128, EC, DC], MM_DT)
    wv_t = sb_big.tile([128, EC, DC], MM_DT)
    wo_t = sb_big.tile([128, 2, E], MM_DT)
    bq_t = sb_big.tile([128, 2], F32)
    bk_t = sb_big.tile([128, 2], F32)
    ones32 = sb_big.tile([128, 32], MM_DT)
    warm = sb_big.tile([1, 8], F32)
    qT_t = sb_big.tile([128, 2, S], MM_DT)
    kT_t = sb_big.tile([128, 2, S], MM_DT)
    v_t = sb_big.tile([128, KC, 2, 128], MM_DT)
    o_t = sb_big.tile([128, 2, S], MM_DT)

    # ACT table preload: dummy exp so the ~2.7us table DMA runs during input
    # DMA rather than before the first real ACTIVATE.
    nc.vector.memset(warm[:, :], 0.0)
    nc.scalar.activation(out=warm[:, :], in_=warm[:, :], func=EXP_FUNC)
    nc.vector.memset(ones32[:, :], 1.0)

    # Input DMAs.  Scalar ring: biases, then wv (needed by the V projection
    # that overlaps this DMA), wq/wk, the tail xT chunks, wo.  Sync ring:
    # head xT chunks.  Both rings drain in parallel.
    nc.scalar.dma_start(out=bq_t[:, :], in_=bq)
    nc.scalar.dma_start(out=bk_t[:, :], in_=bk)
    nc.scalar.dma_start(out=wv_t[:, :, :],
                        in_=wv.rearrange("p (c d) -> p c d", c=EC))
    nc.scalar.dma_start(out=wq_t[:, :, :],
                        in_=wq.rearrange("p (c d) -> p c d", c=EC))
    nc.scalar.dma_start(out=wk_t[:, :, :],
                        in_=wk.rearrange("p (c d) -> p c d", c=EC))
    for ec in range(EC):
        eng = nc.sync if ec < 4 else nc.scalar
        eng.dma_start(out=xT_t[:, ec, :], in_=xT[:, ec * S:(ec + 1) * S])
    nc.scalar.dma_start(out=wo_t[:, :, :],
                        in_=wo.rearrange("p (c e) -> p c e", c=2))

    # ---- startup: V projection (kc pairs 0-5) + kT(0,s0) + qT(0,0), all
    # ec-interleaved so matmuls start as xT chunks land.  Bank map during
    # startup: ringA -> kc pairs 0-2, ringB -> 3-4, acc -> 5,
    # qk-bank -> kT(0,s0), y-bank -> qT(0,0).  kc pairs 6-7 are early
    # stream fillers (needed by PV only near the end of block 0).
    ringA0 = ps_ring.tile([128, 3, 512], F32, tag="A", name="v_ringA")
    ringB0 = ps_ring.tile([128, 2, 512], F32, tag="B", name="v_ringB")
    accV = ps_acc.tile([128, 512], F32, tag="bank", name="v_acc")
    qk_ps0 = ps_qk.tile([128, 512], F32, tag="bank", name="ps_k00")
    y_ps0 = ps_y.tile([128, 512], F32, tag="bank", name="ps_q00")

    def v_bank(kcp):
        if kcp < 3:
            return ringA0[:, kcp, :]
        if kcp < 5:
            return ringB0[:, kcp - 3, :]
        return accV[:, :]

    def v_mms(ec, kcp):
        for half in range(2):
            kc = 2 * kcp + half
            nc.tensor.matmul(
                v_bank(kcp)[:, half * 256:half * 256 + 256],
                lhsT=xT_t[:, ec, kc * 128:(kc + 1) * 128],
                rhs=wv_t[:, ec, :],
                start=(ec == 0 and half == 0), stop=(ec == EC - 1))

    for ec in range(EC):
        for kcp in range(6):
            v_mms(ec, kcp)
        nc.tensor.matmul(   # kT(0, s0)
            qk_ps0[:, :], lhsT=wk_t[:, ec, 0:128],
            rhs=xT_t[:, ec, 0:512], start=(ec == 0), stop=(ec == EC - 1))
        nc.tensor.matmul(   # qT(0, 0)
            y_ps0[:, :], lhsT=wq_t[:, ec, 0:128],
            rhs=xT_t[:, ec, 0:512], start=(ec == 0), stop=(ec == EC - 1))
    nc.vector.tensor_scalar_add(out=kT_t[:, 0, 0:512], in0=qk_ps0[:, :],
                                scalar1=bk_t[:, 0:1])
    nc.vector.tensor_scalar_add(out=qT_t[:, 0, 0:512], in0=y_ps0[:, :],
                                scalar1=bq_t[:, 0:1])
    for kcp in range(6):
        for half in range(2):
            kc = 2 * kcp + half
            nc.vector.tensor_copy(
                out=v_t[:, kc, :, :],
                in_=v_bank(kcp)[:, half * 256:half * 256 + 256].rearrange(
                    "p (h d) -> p h d", h=2))

    # ---- helpers -------------------------------------------------------
    def qk_half(hc, proj, sc, half, state={}):
        w_t, dst, b_t = ((wq_t, qT_t, bq_t), (wk_t, kT_t, bk_t))[proj]
        key = (hc, proj, sc)
        if half == 0:
            state[key] = ps_qk.tile([128, 512], F32, tag="bank", name="ps_qk")
        ps = state[key]
        for ec in (range(EC // 2) if half == 0 else range(EC // 2, EC)):
            nc.tensor.matmul(
                ps[:, :],
                lhsT=w_t[:, ec, hc * 128:(hc + 1) * 128],
                rhs=xT_t[:, ec, sc * 512:(sc + 1) * 512],
                start=(ec == 0), stop=(ec == EC - 1))
        if half == 1:
            nc.vector.tensor_scalar_add(
                out=dst[:, hc, sc * 512:(sc + 1) * 512],
                in0=ps[:, :], scalar1=b_t[:, hc:hc + 1])
            del state[key]

    def v_fill(kcp, half):
        # late V chunks (kc pairs 6-7) on the qk bank as stream fillers
        key = ("vf", kcp)
        st = v_fill.state
        if half == 0:
            st[key] = ps_qk.tile([128, 512], F32, tag="bank", name="ps_vf")
        ps = st[key]
        kc = 2 * kcp + half
        for ec in range(EC):
            nc.tensor.matmul(
                ps[:, half * 256:half * 256 + 256],
                lhsT=xT_t[:, ec, kc * 128:(kc + 1) * 128],
                rhs=wv_t[:, ec, :],
                start=(ec == 0 and half == 0), stop=(ec == EC - 1))
        nc.vector.tensor_copy(
            out=v_t[:, kc, :, :],
            in_=ps[:, half * 256:half * 256 + 256].rearrange(
                "p (h d) -> p h d", h=2))
        if half == 1:
            del st[key]
    v_fill.state = {}

    def y_half(qc, hc, ec):
        yp = ps_y.tile([128, 512], F32, tag="bank", name="ps_y")
        nc.tensor.matmul(
            yp[:, :],
            lhsT=wo_t[:, hc, ec * 128:(ec + 1) * 128],
            rhs=o_t[:, hc, qc * 512:(qc + 1) * 512],
            start=True, stop=True)
        ys = sb_y.tile([128, 512], MM_DT)
        nc.vector.tensor_copy(out=ys[:, :], in_=yp[:, :])
        nc.sync.dma_start(
            out=yT[hc * E + ec * 128:hc * E + (ec + 1) * 128,
                   qc * 512:(qc + 1) * 512],
            in_=ys[:, :])

    pt_ap = [None] * NSLICE

    def scores_mm(ring, j, p, hp):
        _, kc, qc, hc = pair_info(p)
        po = hp * 64
        nc.tensor.matmul(
            ring[:, j, :],
            lhsT=kT_t[po:po + 64, hc, kc * 128:(kc + 1) * 128],
            rhs=qT_t[po:po + 64, hc, qc * 512:(qc + 1) * 512],
            start=True, stop=True)

    acc_state = {}

    def pv_pair(p):
        b, kc, qc, hc = pair_info(p)
        if b not in acc_state:
            acc_state[b] = ps_acc.tile([128, 512], F32, tag="bank", name="acc")
        acc = acc_state[b]
        for hp in range(2):
            pt, j = pt_ap[2 * p + hp]
            nc.tensor.matmul(
                acc[hp * 64:hp * 64 + 64, :],
                lhsT=v_t[:, kc, hc, hp * 64:hp * 64 + 64],
                rhs=pt[:, j, :],
                start=(kc == 0), stop=(kc == KC - 1))

    dn_state = {}

    def denom_quads(b, q0, nq):
        # lanes: 0 = hp0/kc0-7, 1 = hp0/kc8-15, 2 = hp1/kc0-7, 3 = hp1/kc8-15
        if b not in dn_state:
            dn_state[b] = ps_qk.tile([128, 512], F32, tag="bank", name="ps_dn")
        dn = dn_state[b]
        for step in range(q0, q0 + nq):
            for lane in range(4):
                hp, hkc = lane // 2, lane % 2
                kc = hkc * 8 + step
                pt, j = pt_ap[2 * (b * KC + kc) + hp]
                nc.tensor.matmul(
                    dn[32 * lane:32 * lane + 32, :],
                    lhsT=ones32[:, :],
                    rhs=pt[:, j, :],
                    start=(step == 0), stop=(step == 7),
                    tile_position=(0, 32 * lane))

    def norm_block(b):
        qc, hc = BLOCK_ORDER[b]
        acc = acc_state.pop(b)
        dn = dn_state.pop(b)
        tmp = sb_norm.tile([128, 512], F32, tag="tmp")
        inv = sb_norm.tile([128, 512], F32, tag="inv")
        nc.vector.tensor_copy(out=tmp[:, :], in_=dn[:, :])
        if dbg is not None:
            nc.sync.dma_start(out=dbg["dn"][b * 128:(b + 1) * 128, :],
                              in_=tmp[:, :])
        # r_hp = lane(2hp) + lane(2hp+1), replicated into both 32-row strips.
        # Mixed PSUM+SBUF operands may use different base partitions (the
        # equal-base rule only binds SB+SB pairs).
        for hp in range(2):
            base = 64 * hp
            for sub in range(2):
                nc.vector.tensor_add(
                    out=inv[base + 32 * sub:base + 32 * sub + 32, :],
                    in0=dn[base + 32 * (1 - sub):base + 32 * (2 - sub), :],
                    in1=tmp[base + 32 * sub:base + 32 * sub + 32, :])
        nc.vector.reciprocal_approx_fast(out=inv[:, :], in_=inv[:, :])
        nc.vector.tensor_mul(
            o_t[:, hc, qc * 512:(qc + 1) * 512], acc[:, :], inv[:, :])

    # ---- filler schedule keyed by exp index -----------------------------
    fillers = {}

    def put_slice(dslice, fn):
        # schedule fn at the exp whose cumulative slice count reaches
        # dslice - 12 (approx 1.6us of lookahead)
        tgt = max(0, dslice - 12)
        for e, c in enumerate(CUM_SLICES):
            if c >= tgt:
                fillers.setdefault(e, []).append(fn)
                return
        fillers.setdefault(NEXP - 1, []).append(fn)

    def qkf(hc, proj, sc, half):
        return lambda: qk_half(hc, proj, sc, half)

    # late V fills: needed by PV of block 0 at kc 12..15 (slices 24..31,
    # consumed with ~2-exp lag) — schedule asap
    put_slice(6, lambda: v_fill(6, 0))
    put_slice(9, lambda: v_fill(6, 1))
    put_slice(12, lambda: v_fill(7, 0))
    put_slice(15, lambda: v_fill(7, 1))
    # qk groups: kT(hc,sc) first used at slice 32*hc + 8*sc (hc-inner order);
    # qT(hc,qc) at slice 32*(2qc+hc).  Halves ~3 slices apart.
    qk_seq = [
        (8, 0, 1, 1), (16, 0, 1, 2), (24, 0, 1, 3),
        (32, 1, 1, 0), (34, 1, 0, 0),
        (40, 1, 1, 1), (48, 1, 1, 2), (56, 1, 1, 3),
        (64, 0, 0, 1), (70, 1, 0, 1),
        (128, 0, 0, 2), (134, 1, 0, 2),
        (192, 0, 0, 3), (198, 1, 0, 3),
    ]
    for dsl, hc, proj, sc in qk_seq:
        put_slice(dsl - 3, qkf(hc, proj, sc, 0))
        put_slice(dsl, qkf(hc, proj, sc, 1))

    # ---- main stream ----------------------------------------------------
    pv_done = 0
    dn_done = [0] * NBLK
    normed = [False] * NBLK
    y_queue = []

    def emit_background(e):
        nonlocal pv_done
        ready = CUM_SLICES[e - 1] if e >= 1 else 0   # slices in exps <= e-1
        budget = 3
        while budget > 0 and pv_done < NPAIR:
            if 2 * (pv_done + 1) > ready:
                break
            b, kc = pv_done // KC, pv_done % KC
            if kc == 0 and b > 0 and not normed[b - 1]:
                break
            pv_pair(pv_done)
            pv_done += 1
            budget -= 1
        for b in range(NBLK):
            if dn_done[b] < 8 and 2 * (b + 1) * PAIRS_PER_BLOCK <= ready:
                nq = min(2, 8 - dn_done[b])
                denom_quads(b, dn_done[b], nq)
                dn_done[b] += nq
                break
        for b in range(NBLK):
            if dn_done[b] == 8 and not normed[b] and pv_done >= (b + 1) * KC:
                norm_block(b)
                normed[b] = True
                qc, hc = BLOCK_ORDER[b]
                for ec in range(EC):
                    y_queue.append((qc, hc, ec))
                break
        if y_queue:
            y_half(*y_queue.pop(0))

    # slice -> (exp tile index, position)
    slice_pos = {}
    for e, (tag, slices) in enumerate(EXP_TILES):
        for j, (p, hp) in enumerate(slices):
            slice_pos[2 * p + hp] = (e, j)

    # Pair-by-pair emission: both matmuls of a pair are always adjacent
    # (concurrent 64-row PE tiles); each exp fires as soon as its slices are
    # emitted.  Ring/pt tiles allocate lazily at their first slice.
    ring_cur = {}
    next_exp = 0

    def flush_exps(pairs_done):
        nonlocal next_exp
        while next_exp < NEXP and CUM_SLICES[next_exp] <= 2 * pairs_done:
            tag, slices = EXP_TILES[next_exp]
            ring, pt = ring_cur.pop(next_exp)
            n = len(slices)
            nc.scalar.activation(
                out=pt[:, 0:n, :], in_=ring[:, 0:n, :], func=EXP_FUNC,
                scale=float(SCALE))
            for fn in fillers.get(next_exp, []):
                fn()
            emit_background(next_exp)
            next_exp += 1

    for p in range(NPAIR):
        for hp in range(2):
            s = 2 * p + hp
            e, j = slice_pos[s]
            if e not in ring_cur:
                tag = EXP_TILES[e][0]
                shape = [128, 3 if tag == "A" else 2, 512]
                ring = ps_ring.tile(shape, F32, tag=tag, name=f"ring{tag}")
                pt = sb_p.tile(shape, MM_DT, tag=f"pt{tag}", name=f"pt{tag}")
                ring_cur[e] = (ring, pt)
            ring, pt = ring_cur[e]
            scores_mm(ring, j, p, hp)
            pt_ap[s] = (pt, j)
        flush_exps(p + 1)

    # ---- drain ----------------------------------------------------------
    e = NEXP + 1
    while pv_done < NPAIR or not all(normed) or y_queue:
        emit_background(min(e, NEXP))
        e += 1
        if e > NEXP + 300:
            raise RuntimeError("drain did not converge")

    if dbg is not None:
        for name, src in (("qT", qT_t), ("kT", kT_t), ("o", o_t)):
            nc.sync.dma_start(out=dbg[name],
                              in_=src.rearrange("p a b -> p (a b)"))
        nc.sync.dma_start(out=dbg["v"],
                          in_=v_t.rearrange("p a b c -> p (a b c)"))


_cached_nc = None


def _build():
    nc = bacc.Bacc(trn_type="TRN2", target_bir_lowering=False)
    xT = nc.dram_tensor("xT", [128, EC * S], MM_DT, kind="ExternalInput").ap()
    wq = nc.dram_tensor("wq", [128, EC * DC], MM_DT, kind="ExternalInput").ap()
    wk = nc.dram_tensor("wk", [128, EC * DC], MM_DT, kind="ExternalInput").ap()
    wv = nc.dram_tensor("wv", [128, EC * DC], MM_DT, kind="ExternalInput").ap()
    wo = nc.dram_tensor("wo", [128, 2 * E], MM_DT, kind="ExternalInput").ap()
    bq = nc.dram_tensor("bq", [128, 2], F32, kind="ExternalInput").ap()
    bk = nc.dram_tensor("bk", [128, 2], F32, kind="ExternalInput").ap()
    yT = nc.dram_tensor("yT", [2 * E, S], MM_DT, kind="ExternalOutput").ap()
    dbg = None
    if DEBUG_DUMPS:
        dbg = {
            "qT": nc.dram_tensor("dbg_qT", [128, 2 * S], MM_DT,
                                 kind="ExternalOutput").ap(),
            "kT": nc.dram_tensor("dbg_kT", [128, 2 * S], MM_DT,
                                 kind="ExternalOutput").ap(),
            "o": nc.dram_tensor("dbg_o", [128, 2 * S], MM_DT,
                                kind="ExternalOutput").ap(),
            "v": nc.dram_tensor("dbg_v", [128, KC * 2 * 128], MM_DT,
                                kind="ExternalOutput").ap(),
            "dn": nc.dram_tensor("dbg_dn", [NBLK * 128, 512], F32,
                                 kind="ExternalOutput").ap(),
        }
    with tile.TileContext(nc) as tc:
        with ExitStack() as ctx:
            _emit(nc, tc, ctx, xT, wq, wk, wv, wo, bq, bk, yT, dbg)
    nc.compile()
    return nc


def get_nc():
    global _cached_nc
    if _cached_nc is None:
        _cached_nc = _build()
    return _cached_nc


def perm(a):
    # [C*128, N] -> [128, C*N] with SBUF chunk-major free dim
    cN = a.shape[0] // 128
    return np.ascontiguousarray(
        a.reshape(cN, 128, a.shape[1]).transpose(1, 0, 2).reshape(
            128, cN * a.shape[1]))


def make_in_maps(inputs, wq, bq, wk, bk, wv, wo):
    in_maps = []
    for c in range(NCORES):
        b, g = divmod(c, GH)
        sl = slice(g * DC, (g + 1) * DC)
        in_maps.append({
            "xT": round_f32r(perm(np.ascontiguousarray(inputs[b].T))),
            "wq": round_f32r(perm(wq[:, sl])),
            "wk": round_f32r(perm(wk[:, sl])),
            "wv": round_f32r(perm(wv[:, sl])),
            "wo": round_f32r(perm(wo[sl, :])),
            "bq": np.ascontiguousarray(bq[sl].reshape(2, 128).T, np.float32),
            "bk": np.ascontiguousarray(bk[sl].reshape(2, 128).T, np.float32),
        })
    return in_maps


def combine(results, wv_full, bv, wo_full, bo):
    y = np.zeros((B, S, E), np.float32)
    for c in range(NCORES):
        yTr = np.asarray(results[c]["yT"], np.float32)
        y[c // GH] += (yTr[:E] + yTr[E:]).T
    y += bv @ wo_full + bo
    return y


def kernel(inputs, wq, bq, wk, bk, wv, bv, wo, bo, _run_kwargs=None):
    inputs = np.asarray(inputs, np.float32)
    wq, bq = np.asarray(wq, np.float32), np.asarray(bq, np.float32)
    wk, bk = np.asarray(wk, np.float32), np.asarray(bk, np.float32)
    wv, bv = np.asarray(wv, np.float32), np.asarray(bv, np.float32)
    wo, bo = np.asarray(wo, np.float32), np.asarray(bo, np.float32)

    nc = get_nc()
    in_maps = make_in_maps(inputs, wq, bq, wk, bk, wv, wo)
    res = run_bass_kernel_spmd(nc, in_maps, list(range(NCORES)),
                               **(_run_kwargs or {}))
    y = combine(res.results, wv, bv, wo, bo)
    if _run_kwargs is not None:
        kernel.last_result = res
    return y


# revision 23
# speedup vs baseline: 1.0404x; 1.0284x over previous
"""Multi-head self-attention (B=2, S=2048, E=1024, H=16, D=64) on 8 NeuronCores.

Sharding: core c -> (batch b = c // 4, head group g = c % 4).  Each core
computes Q/K/V projections for its 4 heads (column-parallel), attention, and
per-head-pair partial output projections (row-parallel); the host sums the 8
partials per batch.  Device activations live in "transposed space" (feature
on the partition dim) so every matmul contracts along partitions:

  Q^T = Wq_g^T @ X^T          [256, 2048]  (bias folded into the psum copy)
  K^T = Wk_g^T @ X^T          [256, 2048]
  V   = X @ Wv_g              [2048, 256]  (mostly projected during input DMA)
  S^T = K_h @ Q_h^T           per head, row-packed head pairs
  P^T = exp(S^T * scale)      ACTIVATEs over 3- and 2-bank psum tiles
  O'^T = V2^T @ P^T           col-packed pair (concurrent 64-col tiles)
  r    = ones32^T @ P^T       col-tiled M=32 matmuls -> denominators
                              replicated over 32 partitions (no broadcast)
  O^T  = O'^T * (1/r)         DVE reciprocal + one full-width multiply
  Y^T[hc] = Wo_hc^T @ O^T[hc] [1024, 2048] fp16 per-pair partial

Schedule: a software pipeline driven by the ScalarE exp stream.  The exp ring
is an asymmetric 5-bank pair [A=3 banks, B=2 banks]; score pairs are permuted
so both matmuls of every (kc, head-pair) land adjacent in emission order and
run as concurrent 64-row PE tiles.  The other 3 psum banks are dedicated:
PV accumulator / qk-projection+denominators / output-projection chunks, so
no PE matmul ever head-blocks on an unrelated psum copy.  start=True clears
the whole psum bank row, so only the first matmul emitted per bank carries
it when two column-half groups share a bank.
"""

from contextlib import ExitStack

import numpy as np

import concourse.bass as bass
import concourse.tile as tile
from concourse import bacc, mybir
from concourse.bass_utils import run_bass_kernel_spmd

B, S, E, H, D = 2, 2048, 1024, 16, 64
NCORES = 8
GH = 4            # heads per core
DC = GH * D       # head-dim columns per core (256)
EC = E // 128     # 8 e-chunks
KC = S // 128     # 16 k-chunks
QC = S // 512     # 4 q-chunks
F32 = mybir.dt.float32
MM_DT = mybir.dt.float16
EXP_FUNC = mybir.ActivationFunctionType.Exp
SCALE = 1.0 / np.sqrt(np.float32(D))

# (qc, hc) block order: hc-inner so y(qc) partials become eligible early and
# the drain only carries the last half-block's output chunks.
BLOCK_ORDER = [(0, 0), (1, 0), (0, 1), (1, 1), (2, 0), (2, 1), (3, 0), (3, 1)]
NBLK = len(BLOCK_ORDER)
PAIRS_PER_BLOCK = KC
NPAIR = NBLK * PAIRS_PER_BLOCK     # 128 (block-major, kc-minor)
NSLICE = 2 * NPAIR

# Exp-ring windows: 5 pairs -> ring tiles [A(3 slices), B(2), A(3), B(2)]
# with pair p emitted adjacently: A=(p0,p0,p1) B=(p1,p2) A=(p2,p3,p3) B=(p4,p4)
WINDOW_PAIRS = 5


def round_f32r(a):
    return np.ascontiguousarray(a, np.float32).astype(np.float16)


def pair_info(p):
    b = p // PAIRS_PER_BLOCK
    kc = p % PAIRS_PER_BLOCK
    qc, hc = BLOCK_ORDER[b]
    return b, kc, qc, hc


def build_windows():
    """Yield exp-tile descriptors: (tag, [(pair, hp), ...]) in emission order.

    Every pair's two slices are adjacent in the global emission sequence,
    possibly straddling two consecutive tiles (safe: by emission time the
    earlier tile of the same tag has long been consumed).
    """
    seq = [(p, hp) for p in range(NPAIR) for hp in range(2)]
    tiles = []
    i = 0
    sizes = [3, 2]
    k = 0
    while i < len(seq):
        n = min(sizes[k % 2], len(seq) - i)
        tiles.append(("A" if k % 2 == 0 else "B", seq[i:i + n]))
        i += n
        k += 1
    return tiles


EXP_TILES = build_windows()
NEXP = len(EXP_TILES)
# cumulative slices after each exp tile
CUM_SLICES = []
_c = 0
for _tag, _sl in EXP_TILES:
    _c += len(_sl)
    CUM_SLICES.append(_c)


DEBUG_DUMPS = False


def _emit(nc, tc, ctx, xT, wq, wk, wv, wo, bq, bk, yT, dbg=None):
    sb_big = ctx.enter_context(tc.tile_pool(name="sb_big", bufs=1))
    sb_p = ctx.enter_context(tc.tile_pool(name="sb_p", bufs=10))
    sb_norm = ctx.enter_context(tc.tile_pool(name="sb_norm", bufs=2))
    sb_y = ctx.enter_context(tc.tile_pool(name="sb_y", bufs=4))
    ps_ring = ctx.enter_context(tc.tile_pool(name="ps_ring", bufs=1, space="PSUM"))
    ps_acc = ctx.enter_context(tc.tile_pool(name="ps_acc", bufs=1, space="PSUM"))
    ps_qk = ctx.enter_context(tc.tile_pool(name="ps_qk", bufs=1, space="PSUM"))
    ps_y = ctx.enter_context(tc.tile_pool(name="ps_y", bufs=1, space="PSUM"))

    xT_t = sb_big.tile([128, EC, S], MM_DT)
    wq_t = sb_big.tile([128, EC, DC], MM_DT)
    wk_t = sb_big.tile([128, EC, DC], MM_DT)
    wv_t = sb_big.tile([128, EC, DC], MM_DT)
    wo_t = sb_big.tile([128, 2, E], MM_DT)
    bq_t = sb_big.tile([128, 2], F32)
    bk_t = sb_big.tile([128, 2], F32)
    ones32 = sb_big.tile([128, 32], MM_DT)
    warm = sb_big.tile([1, 8], F32)
    qT_t = sb_big.tile([128, 2, S], MM_DT)
    kT_t = sb_big.tile([128, 2, S], MM_DT)
    v_t = sb_big.tile([128, KC, 2, 128], MM_DT)
    o_t = sb_big.tile([128, 2, S], MM_DT)

    # Input DMAs.  Scalar ring: biases, then wv (needed by the V projection
    # that overlaps this DMA), wq/wk, the tail xT chunks, wo.  Sync ring:
    # head xT chunks.  Both rings drain in parallel.
    nc.scalar.dma_start(out=bq_t[:, :], in_=bq)
    nc.scalar.dma_start(out=bk_t[:, :], in_=bk)
    nc.scalar.dma_start(out=wv_t[:, :, :],
                        in_=wv.rearrange("p (c d) -> p c d", c=EC))
    nc.scalar.dma_start(out=wq_t[:, :, :],
                        in_=wq.rearrange("p (c d) -> p c d", c=EC))
    nc.scalar.dma_start(out=wk_t[:, :, :],
                        in_=wk.rearrange("p (c d) -> p c d", c=EC))
    for ec in range(EC):
        eng = nc.sync if ec < 4 else nc.scalar
        eng.dma_start(out=xT_t[:, ec, :], in_=xT[:, ec * S:(ec + 1) * S])
    nc.scalar.dma_start(out=wo_t[:, :, :],
                        in_=wo.rearrange("p (c e) -> p c e", c=2))

    # ACT table preload AFTER the dma issues on the scalar queue: exp with
    # scale=0 reads garbage safely (exp(0)) and pulls the ~2.7us table load
    # into the DMA window instead of blocking the ring.
    nc.scalar.activation(out=warm[:, :], in_=warm[:, :], func=EXP_FUNC,
                         scale=0.0)
    nc.vector.memset(ones32[:, :], 1.0)

    # PE pre-warm: a burst of tiny matmuls while the input DMA streams, so
    # the HAM clock gate reaches K=8/8 before the real projections start.
    warm_ps = ps_y.tile([128, 512], F32, tag="bank", name="ps_warm")
    for i in range(64):
        nc.tensor.matmul(warm_ps[0:32, 0:32], lhsT=ones32[:, :],
                         rhs=ones32[:, :], start=(i == 0), stop=(i == 63))

    # ---- startup: V projection (kc pairs 0-5) + kT(0,s0) + qT(0,0), all
    # ec-interleaved so matmuls start as xT chunks land.  Bank map during
    # startup: ringA -> kc pairs 0-2, ringB -> 3-4, acc -> 5,
    # qk-bank -> kT(0,s0), y-bank -> qT(0,0).  kc pairs 6-7 are early
    # stream fillers (needed by PV only near the end of block 0).
    ringA0 = ps_ring.tile([128, 3, 512], F32, tag="A", name="v_ringA")
    ringB0 = ps_ring.tile([128, 2, 512], F32, tag="B", name="v_ringB")
    accV = ps_acc.tile([128, 512], F32, tag="bank", name="v_acc")
    qk_ps0 = ps_qk.tile([128, 512], F32, tag="bank", name="ps_k00")
    y_ps0 = ps_y.tile([128, 512], F32, tag="bank", name="ps_q00")

    def v_bank(kcp):
        if kcp < 3:
            return ringA0[:, kcp, :]
        if kcp < 5:
            return ringB0[:, kcp - 3, :]
        return accV[:, :]

    def v_mms(ec, kcp):
        for half in range(2):
            kc = 2 * kcp + half
            nc.tensor.matmul(
                v_bank(kcp)[:, half * 256:half * 256 + 256],
                lhsT=xT_t[:, ec, kc * 128:(kc + 1) * 128],
                rhs=wv_t[:, ec, :],
                start=(ec == 0 and half == 0), stop=(ec == EC - 1))

    for ec in range(EC):
        for kcp in range(6):
            v_mms(ec, kcp)
        nc.tensor.matmul(   # kT(0, s0)
            qk_ps0[:, :], lhsT=wk_t[:, ec, 0:128],
            rhs=xT_t[:, ec, 0:512], start=(ec == 0), stop=(ec == EC - 1))
        nc.tensor.matmul(   # qT(0, 0)
            y_ps0[:, :], lhsT=wq_t[:, ec, 0:128],
            rhs=xT_t[:, ec, 0:512], start=(ec == 0), stop=(ec == EC - 1))
    nc.vector.tensor_scalar_add(out=kT_t[:, 0, 0:512], in0=qk_ps0[:, :],
                                scalar1=bk_t[:, 0:1])
    nc.vector.tensor_scalar_add(out=qT_t[:, 0, 0:512], in0=y_ps0[:, :],
                                scalar1=bq_t[:, 0:1])
    for kcp in range(6):
        for half in range(2):
            kc = 2 * kcp + half
            nc.vector.tensor_copy(
                out=v_t[:, kc, :, :],
                in_=v_bank(kcp)[:, half * 256:half * 256 + 256].rearrange(
                    "p (h d) -> p h d", h=2))

    # ---- helpers -------------------------------------------------------
    def qk_half(hc, proj, sc, half, state={}):
        w_t, dst, b_t = ((wq_t, qT_t, bq_t), (wk_t, kT_t, bk_t))[proj]
        key = (hc, proj, sc)
        if half == 0:
            state[key] = ps_qk.tile([128, 512], F32, tag="bank", name="ps_qk")
        ps = state[key]
        for ec in (range(EC // 2) if half == 0 else range(EC // 2, EC)):
            nc.tensor.matmul(
                ps[:, :],
                lhsT=w_t[:, ec, hc * 128:(hc + 1) * 128],
                rhs=xT_t[:, ec, sc * 512:(sc + 1) * 512],
                start=(ec == 0), stop=(ec == EC - 1))
        if half == 1:
            nc.vector.tensor_scalar_add(
                out=dst[:, hc, sc * 512:(sc + 1) * 512],
                in0=ps[:, :], scalar1=b_t[:, hc:hc + 1])
            del state[key]

    def v_fill(kcp, half):
        # late V chunks (kc pairs 6-7) on the qk bank as stream fillers
        key = ("vf", kcp)
        st = v_fill.state
        if half == 0:
            st[key] = ps_qk.tile([128, 512], F32, tag="bank", name="ps_vf")
        ps = st[key]
        kc = 2 * kcp + half
        for ec in range(EC):
            nc.tensor.matmul(
                ps[:, half * 256:half * 256 + 256],
                lhsT=xT_t[:, ec, kc * 128:(kc + 1) * 128],
                rhs=wv_t[:, ec, :],
                start=(ec == 0 and half == 0), stop=(ec == EC - 1))
        nc.vector.tensor_copy(
            out=v_t[:, kc, :, :],
            in_=ps[:, half * 256:half * 256 + 256].rearrange(
                "p (h d) -> p h d", h=2))
        if half == 1:
            del st[key]
    v_fill.state = {}

    def y_half(qc, hc, ec, pool=None):
        yp = (pool or ps_y).tile([128, 512], F32, tag="bank", name="ps_y")
        nc.tensor.matmul(
            yp[:, :],
            lhsT=wo_t[:, hc, ec * 128:(ec + 1) * 128],
            rhs=o_t[:, hc, qc * 512:(qc + 1) * 512],
            start=True, stop=True)
        ys = sb_y.tile([128, 512], MM_DT)
        nc.vector.tensor_copy(out=ys[:, :], in_=yp[:, :])
        nc.sync.dma_start(
            out=yT[hc * E + ec * 128:hc * E + (ec + 1) * 128,
                   qc * 512:(qc + 1) * 512],
            in_=ys[:, :])

    pt_ap = [None] * NSLICE

    def scores_mm(ring, j, p, hp):
        _, kc, qc, hc = pair_info(p)
        po = hp * 64
        nc.tensor.matmul(
            ring[:, j, :],
            lhsT=kT_t[po:po + 64, hc, kc * 128:(kc + 1) * 128],
            rhs=qT_t[po:po + 64, hc, qc * 512:(qc + 1) * 512],
            start=True, stop=True)

    acc_state = {}

    def pv_pair(p):
        b, kc, qc, hc = pair_info(p)
        if b not in acc_state:
            acc_state[b] = ps_acc.tile([128, 512], F32, tag="bank", name="acc")
        acc = acc_state[b]
        for hp in range(2):
            pt, j = pt_ap[2 * p + hp]
            nc.tensor.matmul(
                acc[hp * 64:hp * 64 + 64, :],
                lhsT=v_t[:, kc, hc, hp * 64:hp * 64 + 64],
                rhs=pt[:, j, :],
                start=(kc == 0), stop=(kc == KC - 1))

    dn_state = {}

    def denom_quads(b, q0, nq):
        # lanes: 0 = hp0/kc0-7, 1 = hp0/kc8-15, 2 = hp1/kc0-7, 3 = hp1/kc8-15
        if b not in dn_state:
            dn_state[b] = ps_qk.tile([128, 512], F32, tag="bank", name="ps_dn")
        dn = dn_state[b]
        for step in range(q0, q0 + nq):
            for lane in range(4):
                hp, hkc = lane // 2, lane % 2
                kc = hkc * 8 + step
                pt, j = pt_ap[2 * (b * KC + kc) + hp]
                nc.tensor.matmul(
                    dn[32 * lane:32 * lane + 32, :],
                    lhsT=ones32[:, :],
                    rhs=pt[:, j, :],
                    start=(step == 0), stop=(step == 7),
                    tile_position=(0, 32 * lane))

    def norm_block(b):
        qc, hc = BLOCK_ORDER[b]
        acc = acc_state.pop(b)
        dn = dn_state.pop(b)
        tmp = sb_norm.tile([128, 512], F32, tag="tmp")
        inv = sb_norm.tile([128, 512], F32, tag="inv")
        nc.vector.tensor_copy(out=tmp[:, :], in_=dn[:, :])
        if dbg is not None:
            nc.sync.dma_start(out=dbg["dn"][b * 128:(b + 1) * 128, :],
                              in_=tmp[:, :])
        # r_hp = lane(2hp) + lane(2hp+1), replicated into both 32-row strips.
        # Mixed PSUM+SBUF operands may use different base partitions (the
        # equal-base rule only binds SB+SB pairs).
        for hp in range(2):
            base = 64 * hp
            for sub in range(2):
                nc.vector.tensor_add(
                    out=inv[base + 32 * sub:base + 32 * sub + 32, :],
                    in0=dn[base + 32 * (1 - sub):base + 32 * (2 - sub), :],
                    in1=tmp[base + 32 * sub:base + 32 * sub + 32, :])
        nc.vector.reciprocal_approx_fast(out=inv[:, :], in_=inv[:, :])
        nc.vector.tensor_mul(
            o_t[:, hc, qc * 512:(qc + 1) * 512], acc[:, :], inv[:, :])

    # ---- filler schedule keyed by exp index -----------------------------
    fillers = {}

    def put_slice(dslice, fn):
        # schedule fn at the exp whose cumulative slice count reaches
        # dslice - 12 (approx 1.6us of lookahead)
        tgt = max(0, dslice - 12)
        for e, c in enumerate(CUM_SLICES):
            if c >= tgt:
                fillers.setdefault(e, []).append(fn)
                return
        fillers.setdefault(NEXP - 1, []).append(fn)

    def qkf(hc, proj, sc, half):
        return lambda: qk_half(hc, proj, sc, half)

    # late V fills: needed by PV of block 0 at kc 12..15 (slices 24..31,
    # consumed with ~2-exp lag) — schedule asap
    put_slice(6, lambda: v_fill(6, 0))
    put_slice(9, lambda: v_fill(6, 1))
    put_slice(12, lambda: v_fill(7, 0))
    put_slice(15, lambda: v_fill(7, 1))
    # qk groups: kT(hc,sc) first used at slice 32*hc + 8*sc (hc-inner order);
    # qT(hc,qc) at slice 32*(2qc+hc).  Halves ~3 slices apart.
    qk_seq = [
        (8, 0, 1, 1), (16, 0, 1, 2), (24, 0, 1, 3),
        (32, 0, 0, 1),
        (64, 1, 1, 0), (66, 1, 0, 0),
        (72, 1, 1, 1), (80, 1, 1, 2), (88, 1, 1, 3),
        (96, 1, 0, 1),
        (128, 0, 0, 2), (160, 1, 0, 2),
        (192, 0, 0, 3), (224, 1, 0, 3),
    ]
    for dsl, hc, proj, sc in qk_seq:
        put_slice(dsl - 3, qkf(hc, proj, sc, 0))
        put_slice(dsl, qkf(hc, proj, sc, 1))

    # ---- main stream ----------------------------------------------------
    pv_done = 0
    dn_done = [0] * NBLK
    normed = [False] * NBLK
    y_queue = []

    def emit_background(e):
        nonlocal pv_done
        ready = CUM_SLICES[e - 1] if e >= 1 else 0   # slices in exps <= e-1
        budget = 3
        while budget > 0 and pv_done < NPAIR:
            if 2 * (pv_done + 1) > ready:
                break
            b, kc = pv_done // KC, pv_done % KC
            if kc == 0 and b > 0 and not normed[b - 1]:
                break
            pv_pair(pv_done)
            pv_done += 1
            budget -= 1
        for b in range(NBLK):
            if dn_done[b] < 8 and 2 * (b + 1) * PAIRS_PER_BLOCK <= ready:
                nq = min(2, 8 - dn_done[b])
                denom_quads(b, dn_done[b], nq)
                dn_done[b] += nq
                break
        for b in range(NBLK):
            if dn_done[b] == 8 and not normed[b] and pv_done >= (b + 1) * KC:
                norm_block(b)
                normed[b] = True
                qc, hc = BLOCK_ORDER[b]
                for ec in range(EC):
                    y_queue.append((qc, hc, ec))
                break
        if y_queue:
            y_half(*y_queue.pop(0))


    # slice -> (exp tile index, position)
    slice_pos = {}
    for e, (tag, slices) in enumerate(EXP_TILES):
        for j, (p, hp) in enumerate(slices):
            slice_pos[2 * p + hp] = (e, j)

    # Pair-by-pair emission: both matmuls of a pair are always adjacent
    # (concurrent 64-row PE tiles); each exp fires as soon as its slices are
    # emitted.  Ring/pt tiles allocate lazily at their first slice.
    ring_cur = {}
    next_exp = 0

    def flush_exps(pairs_done):
        nonlocal next_exp
        while next_exp < NEXP and CUM_SLICES[next_exp] <= 2 * pairs_done:
            tag, slices = EXP_TILES[next_exp]
            ring, pt = ring_cur.pop(next_exp)
            n = len(slices)
            nc.scalar.activation(
                out=pt[:, 0:n, :], in_=ring[:, 0:n, :], func=EXP_FUNC,
                scale=float(SCALE))
            for fn in fillers.get(next_exp, []):
                fn()
            emit_background(next_exp)
            next_exp += 1

    for p in range(NPAIR):
        for hp in range(2):
            s = 2 * p + hp
            e, j = slice_pos[s]
            if e not in ring_cur:
                tag = EXP_TILES[e][0]
                shape = [128, 3 if tag == "A" else 2, 512]
                ring = ps_ring.tile(shape, F32, tag=tag, name=f"ring{tag}")
                pt = sb_p.tile(shape, MM_DT, tag=f"pt{tag}", name=f"pt{tag}")
                ring_cur[e] = (ring, pt)
            ring, pt = ring_cur[e]
            scores_mm(ring, j, p, hp)
            pt_ap[s] = (pt, j)
        flush_exps(p + 1)

    # ---- drain ----------------------------------------------------------
    e = NEXP + 1
    drain_i = 0
    while pv_done < NPAIR or not all(normed):
        emit_background(min(e, NEXP))
        e += 1
        if e > NEXP + 300:
            raise RuntimeError("drain did not converge")
    while y_queue:
        y_half(*y_queue.pop(0), pool=(ps_y if drain_i % 2 == 0 else ps_qk))
        drain_i += 1

    if dbg is not None:
        for name, src in (("qT", qT_t), ("kT", kT_t), ("o", o_t)):
            nc.sync.dma_start(out=dbg[name],
                              in_=src.rearrange("p a b -> p (a b)"))
        nc.sync.dma_start(out=dbg["v"],
                          in_=v_t.rearrange("p a b c -> p (a b c)"))


_cached_nc = None


def _build():
    nc = bacc.Bacc(trn_type="TRN2", target_bir_lowering=False)
    xT = nc.dram_tensor("xT", [128, EC * S], MM_DT, kind="ExternalInput").ap()
    wq = nc.dram_tensor("wq", [128, EC * DC], MM_DT, kind="ExternalInput").ap()
    wk = nc.dram_tensor("wk", [128, EC * DC], MM_DT, kind="ExternalInput").ap()
    wv = nc.dram_tensor("wv", [128, EC * DC], MM_DT, kind="ExternalInput").ap()
    wo = nc.dram_tensor("wo", [128, 2 * E], MM_DT, kind="ExternalInput").ap()
    bq = nc.dram_tensor("bq", [128, 2], F32, kind="ExternalInput").ap()
    bk = nc.dram_tensor("bk", [128, 2], F32, kind="ExternalInput").ap()
    yT = nc.dram_tensor("yT", [2 * E, S], MM_DT, kind="ExternalOutput").ap()
    dbg = None
    if DEBUG_DUMPS:
        dbg = {
            "qT": nc.dram_tensor("dbg_qT", [128, 2 * S], MM_DT,
                                 kind="ExternalOutput").ap(),
            "kT": nc.dram_tensor("dbg_kT", [128, 2 * S], MM_DT,
                                 kind="ExternalOutput").ap(),
            "o": nc.dram_tensor("dbg_o", [128, 2 * S], MM_DT,
                                kind="ExternalOutput").ap(),
            "v": nc.dram_tensor("dbg_v", [128, KC * 2 * 128], MM_DT,
                                kind="ExternalOutput").ap(),
            "dn": nc.dram_tensor("dbg_dn", [NBLK * 128, 512], F32,
                                 kind="ExternalOutput").ap(),
        }
    with tile.TileContext(nc) as tc:
        with ExitStack() as ctx:
            _emit(nc, tc, ctx, xT, wq, wk, wv, wo, bq, bk, yT, dbg)
    nc.compile()
    return nc


def get_nc():
    global _cached_nc
    if _cached_nc is None:
        _cached_nc = _build()
    return _cached_nc


def perm(a):
    # [C*128, N] -> [128, C*N] with SBUF chunk-major free dim
    cN = a.shape[0] // 128
    return np.ascontiguousarray(
        a.reshape(cN, 128, a.shape[1]).transpose(1, 0, 2).reshape(
            128, cN * a.shape[1]))


def make_in_maps(inputs, wq, bq, wk, bk, wv, wo):
    in_maps = []
    for c in range(NCORES):
        b, g = divmod(c, GH)
        sl = slice(g * DC, (g + 1) * DC)
        in_maps.append({
            "xT": round_f32r(perm(np.ascontiguousarray(inputs[b].T))),
            "wq": round_f32r(perm(wq[:, sl])),
            "wk": round_f32r(perm(wk[:, sl])),
            "wv": round_f32r(perm(wv[:, sl])),
            "wo": round_f32r(perm(wo[sl, :])),
            "bq": np.ascontiguousarray(bq[sl].reshape(2, 128).T, np.float32),
            "bk": np.ascontiguousarray(bk[sl].reshape(2, 128).T, np.float32),
        })
    return in_maps


def combine(results, wv_full, bv, wo_full, bo):
    y = np.zeros((B, S, E), np.float32)
    for c in range(NCORES):
        yTr = np.asarray(results[c]["yT"], np.float32)
        y[c // GH] += (yTr[:E] + yTr[E:]).T
    y += bv @ wo_full + bo
    return y


def kernel(inputs, wq, bq, wk, bk, wv, bv, wo, bo, _run_kwargs=None):
    inputs = np.asarray(inputs, np.float32)
    wq, bq = np.asarray(wq, np.float32), np.asarray(bq, np.float32)
    wk, bk = np.asarray(wk, np.float32), np.asarray(bk, np.float32)
    wv, bv = np.asarray(wv, np.float32), np.asarray(bv, np.float32)
    wo, bo = np.asarray(wo, np.float32), np.asarray(bo, np.float32)

    nc = get_nc()
    in_maps = make_in_maps(inputs, wq, bq, wk, bk, wv, wo)
    res = run_bass_kernel_spmd(nc, in_maps, list(range(NCORES)),
                               **(_run_kwargs or {}))
    y = combine(res.results, wv, bv, wo, bo)
    if _run_kwargs is not None:
        kernel.last_result = res
    return y
